# revision 14
# baseline (speedup 1.0000x reference)
"""JointGNN message-passing kernel for 8 Trainium2 NeuronCores.

Sharding: nodes in 8 contiguous ranges of N/8; edges sorted by target (row)
so each core owns every edge that aggregates into its nodes (scatter-add is
core-local).  Node state is replicated via a per-layer AllGather (~1.3MB/rank).

Layout: compute is feature-major ([D partitions, edges free]); matmul inputs
bf16, accumulation + state fp32.  The replicated node table lives in SBUF as
bf16 and x_i/x_j are fetched with dma_gather(transpose=True).  Aggregation:
edges are padded per 128-node window to a uniform per-window count, so each
128-edge chunk maps to exactly one window -> one is_equal one-hot + one
matmul per chunk accumulating into that window's PSUM tile; the static
program is identical on all cores (SPMD-safe).
"""
import math
import os
import sys

import numpy as np

for _p in ("/opt/trn_rl_repo", "/root/.axon_site/_ro/trn_rl_repo"):
    if os.path.isdir(_p) and _p not in sys.path:
        sys.path.append(_p)

import ml_dtypes  # noqa: E402

BF16 = ml_dtypes.bfloat16

P = 128
H = 8
NCORES = 8
MACRO = 2048   # edges per gather/load macro-tile
SUB = 512      # edges per compute sub-tile (matmul free dim)
F32 = np.float32


# ----------------------------------------------------------------------------
# host-side data prep
# ----------------------------------------------------------------------------

def _prep(node, edge, geo_feature, edge_index, weights):
    N, D = node.shape
    E = edge.shape[0]
    NS = N // NCORES
    NSp = int(math.ceil(NS / SUB) * SUB)
    NWIN = NSp // P

    row = edge_index[0].astype(np.int64)
    col = edge_index[1].astype(np.int64)
    core_of = row // NS
    perm = np.argsort(row, kind="stable")  # cores are contiguous row ranges

    # per (core, window) counts -> uniform padded window size Ew
    rowloc_all = row - core_of * NS
    win_all = core_of * NWIN + rowloc_all // P          # global window id
    wcounts = np.bincount(win_all, minlength=NCORES * NWIN)
    Ew = int(math.ceil(max(int(wcounts.max()), 1) / P) * P)
    E_body = NWIN * Ew
    E_pad = int(math.ceil(E_body / MACRO) * MACRO)

    cores = []
    sgeo = 1.0 / (1.0 + np.exp(-geo_feature))            # sigmoid(geo)
    geo_w, geo_b = weights["geo_w"], weights["geo_b"]
    cgeo_full = geo_feature @ geo_w[D:, :] + geo_b       # [N, 1]

    for c in range(NCORES):
        sel = perm[core_of[perm] == c]                   # sorted by row
        nloc = NS
        # window-packed padded edge stream
        rl = rowloc_all[sel]
        win = rl // P
        pad_pos = np.full(E_pad, -1, np.int64)           # -1 => pad slot
        pos_of = np.empty(len(sel), np.int64)
        cursor = 0
        for w in range(NWIN):
            m = win == w
            k = int(m.sum())
            idxs = np.nonzero(m)[0]
            pos_of[idxs] = w * Ew + np.arange(k)
            pad_pos[w * Ew: w * Ew + k] = idxs
        valid = pad_pos >= 0
        # per padded slot: source edge (or pad)
        src = pad_pos.copy()

        def take(a, fill):
            out = np.full((E_pad,) + a.shape[1:], fill, a.dtype)
            out[valid] = a[sel][src[valid]]
            return out

        g_row = take(row.astype(np.int32)[:, None], 0)[:, 0]
        g_col = take(col.astype(np.int32)[:, None], 0)[:, 0]
        rloc = np.full(E_pad, -10000.0, F32)
        rloc[valid] = rl[src[valid]].astype(F32)
        eT = np.zeros((P, E_pad), F32)
        eT[:, valid] = edge[sel][src[valid]].T

        nmacro = E_pad // MACRO

        def wrap16(idx):
            a = idx.reshape(nmacro, P, 16)               # [m, c, p]
            w16 = a.transpose(2, 0, 1).reshape(16, nmacro * P)
            return np.tile(w16, (8, 1)).astype(np.int16)

        nodeT = np.zeros((P, NSp), F32)
        nodeT[:, :NS] = node[c * NS:(c + 1) * NS].T
        sgeoT = np.zeros((P, NSp), F32)
        sgeoT[:, :NS] = sgeo[c * NS:(c + 1) * NS].T
        cgeo = np.zeros((1, NSp), F32)
        cgeo[0, :NS] = cgeo_full[c * NS:(c + 1) * NS, 0]

        cores.append(dict(
            edgeT_in=eT,
            idx_row=wrap16(g_row),
            idx_col=wrap16(g_col),
            rowloc=rloc.reshape(-1, P).T.copy(),          # [128, E_pad//128]
            nodeT_in=nodeT,
            sgeoT=sgeoT,
            cgeo_bf=cgeo.astype(BF16),
            _sel=sel, _valid=valid, _src=src,
        ))

    meta = dict(N=N, D=D, E=E, NS=NS, NSp=NSp, NWIN=NWIN, Ew=Ew,
                E_pad=E_pad, KW=Ew // P, perm=perm)
    return cores, meta


def _prep_weights(w, meta):
    """Pre-transform all weights into device layouts (host side)."""
    D = meta["D"]
    L = w["We1"].shape[0]
    dnp = D // H
    temp = math.sqrt(w["Wa2"].shape[-1])
    g = np.arange(D)
    perm_qv = (g % dnp) * H + (g // dnp)    # new col g <- orig col

    out = {}

    def bf(x):
        return np.ascontiguousarray(x).astype(BF16)

    for i in range(L):
        We1 = w["We1"][i]                    # [3D, 2D]
        out[f"We1p_{i}"] = bf(np.hstack([We1[0:D], We1[D:2 * D], We1[2 * D:3 * D]]))  # [D, 3*2D]
        out[f"qkv_{i}"] = bf(np.hstack([
            w["Wq"][i][:, perm_qv], w["Wk"][i][:, perm_qv], w["Wv"][i][:, perm_qv]]))  # [D, 3D]
        Wa1, Wa2 = w["Wa1"][i], w["Wa2"][i]
        A1 = np.zeros((2 * D, 2 * D), F32)
        for h in range(H):
            A1[h * dnp:(h + 1) * dnp, h * 2 * dnp:(h + 1) * 2 * dnp] = Wa1[:dnp]
            A1[D + h * dnp:D + (h + 1) * dnp, h * 2 * dnp:(h + 1) * 2 * dnp] = Wa1[dnp:]
        A2 = np.zeros((2 * D, D), F32)
        for h in range(H):
            A2[h * 2 * dnp:(h + 1) * 2 * dnp, h * dnp:(h + 1) * dnp] = Wa2 / temp
        out[f"Wa1e_{i}"] = bf(A1)
        out[f"Wa2e_{i}"] = bf(A2)
        out[f"We2_{i}"] = bf(w["We2"][i])                 # [2D, D]
        Wu1 = w["Wu1"][i].copy()
        Wu1[D:2 * D] = w["Wu1"][i][D + perm_qv]
        out[f"Wu1p_{i}"] = bf(Wu1)                        # [2D, 2D]
        out[f"Wu2_{i}"] = bf(w["Wu2"][i])                 # [2D, D]

        # f32 bias bank columns for this layer
        out[f"bias_{i}"] = np.stack([
            w["be1"][i][0:D], w["be1"][i][D:2 * D], w["be2"][i],
            w["bq"][i][perm_qv], w["bk"][i][perm_qv],
            np.tile(w["ba1"][i], H)[0:D], np.tile(w["ba1"][i], H)[D:2 * D],
            w["bu1"][i][0:D], w["bu1"][i][D:2 * D], w["bu2"][i],
        ], axis=1).astype(F32)                            # [D, 10]
        out[f"ba2e_{i}"] = bf((np.tile(w["ba2"][i], H) / temp)[None, :])   # [1, D]
        out[f"bve_{i}"] = bf(w["bv"][i][perm_qv][None, :])                 # [1, D]

    for nm in ("gnw_ih", "gnw_hh", "gew_ih", "gew_hh"):
        out[nm + "_bf"] = bf(w[nm])                       # [D, 3D]
    # GRU bias bank: node rc, zc, bhh_n, bih_n; edge same  -> [D, 8]
    gb = []
    for pre in ("gn", "ge"):
        bi, bh = w[pre + "b_ih"], w[pre + "b_hh"]
        gb += [bi[0:D] + bh[0:D], bi[D:2 * D] + bh[D:2 * D], bh[2 * D:], bi[2 * D:]]
    out["gbias"] = np.stack(gb, axis=1).astype(F32)       # [D, 8]
    out["g0_bf"] = bf(w["geo_w"][:D, :])                  # [D, 1]
    out["iota"] = np.broadcast_to(np.arange(P, dtype=F32), (P, P)).copy()
    out["ones1"] = np.ones((1, P), BF16)
    return out


# ----------------------------------------------------------------------------
# device kernel builder
# ----------------------------------------------------------------------------

def _build(meta, L):
    from contextlib import ExitStack
    from concourse import bacc, mybir, tile
    from concourse.masks import make_identity

    D, NSp, NWIN, KW = meta["D"], meta["NSp"], meta["NWIN"], meta["KW"]
    E_pad, NS, N = meta["E_pad"], meta["NS"], meta["N"]
    NMAC = E_pad // MACRO
    NSUBT = NSp // SUB                       # node sub-tiles
    NTpad = int(math.ceil(N / P) * P)
    NSTRIPE = NTpad // P
    dt = mybir.dt
    AF = mybir.ActivationFunctionType
    OP = mybir.AluOpType

    nc = bacc.Bacc("TRN2", target_bir_lowering=False, debug=False,
                   enable_asserts=True, num_devices=NCORES)

    # ---- dram tensors ----
    di = {}

    def din(name, shape, d=dt.float32):
        di[name] = nc.dram_tensor(name, list(shape), d, kind="ExternalInput")
        return di[name]

    din("edgeT_in", [P, E_pad])
    din("idx_row", [P, E_pad // 16], dt.int16)
    din("idx_col", [P, E_pad // 16], dt.int16)
    din("rowloc", [P, E_pad // P])
    din("nodeT_in", [P, NSp])
    din("sgeoT", [P, NSp])
    din("cgeo_bf", [1, NSp], dt.bfloat16)
    for i in range(L):
        din(f"We1p_{i}", [D, 6 * D], dt.bfloat16)
        din(f"qkv_{i}", [D, 3 * D], dt.bfloat16)
        din(f"Wa1e_{i}", [2 * D, 2 * D], dt.bfloat16)
        din(f"Wa2e_{i}", [2 * D, D], dt.bfloat16)
        din(f"We2_{i}", [2 * D, D], dt.bfloat16)
        din(f"Wu1p_{i}", [2 * D, 2 * D], dt.bfloat16)
        din(f"Wu2_{i}", [2 * D, D], dt.bfloat16)
        din(f"bias_{i}", [D, 10])
        din(f"ba2e_{i}", [1, D], dt.bfloat16)
        din(f"bve_{i}", [1, D], dt.bfloat16)
    for nm in ("gnw_ih_bf", "gnw_hh_bf", "gew_ih_bf", "gew_hh_bf"):
        din(nm, [D, 3 * D], dt.bfloat16)
    din("gbias", [D, 8])
    din("g0_bf", [D, 1], dt.bfloat16)
    din("iota", [P, P])
    din("ones1", [1, P], dt.bfloat16)

    node_out = nc.dram_tensor("node_out", [P, NSp], dt.float32, kind="ExternalOutput")
    edge_out = nc.dram_tensor("edge_out", [P, E_pad], dt.float32, kind="ExternalOutput")
    probs_o = [nc.dram_tensor(f"probs{i}", [E_pad // P, P, P], dt.float32,
                              kind="ExternalOutput") for i in range(L)]

    with tile.TileContext(nc) as tc, ExitStack() as ctx:
        wp = ctx.enter_context(tc.tile_pool(name="wp", bufs=1))
        nsp = ctx.enter_context(tc.tile_pool(name="nsp", bufs=1))
        emp = ctx.enter_context(tc.tile_pool(name="emp", bufs=2))
        wkp = ctx.enter_context(tc.tile_pool(name="wkp", bufs=2))
        ckp = ctx.enter_context(tc.tile_pool(name="ckp", bufs=4))
        drp = ctx.enter_context(tc.tile_pool(name="drp", bufs=1, space="DRAM"))
        # PSUM is 8 banks of 2KB; every slot pads to one bank:
        # pbig 4 + psml 2 + pwin 2 = 8 banks.
        pbig = ctx.enter_context(tc.tile_pool(name="pbig", bufs=4, space="PSUM"))
        psml = ctx.enter_context(tc.tile_pool(name="psml", bufs=2, space="PSUM"))
        pwin = ctx.enter_context(tc.tile_pool(name="pwin", bufs=2, space="PSUM"))

        # ---- load constants/weights into SBUF ----
        W = {}
        for nm in list(di.keys()):
            t = di[nm]
            sh = list(t.shape)
            if nm in ("edgeT_in", "idx_row", "idx_col", "rowloc",
                      "nodeT_in", "sgeoT"):
                continue
            if sh[0] > P:  # split into 128-row tiles
                parts = []
                for k in range(sh[0] // P):
                    tt = wp.tile([P, sh[1]], t.dtype, name=f"{nm}_sb{k}",
                                 tag=f"{nm}_sb{k}")
                    nc.sync.dma_start(tt[:], t[k * P:(k + 1) * P, :])
                    parts.append(tt)
                W[nm] = parts
            else:
                tt = wp.tile(sh, t.dtype, name=f"{nm}_sb", tag=f"{nm}_sb")
                nc.sync.dma_start(tt[:], t[:, :])
                W[nm] = tt
        ident = wp.tile([P, P], dt.float32, name="ident", tag="ident")
        make_identity(nc, ident[:])

        def bias(i, j):     # [P,1] f32 column from layer-i bank
            return W[f"bias_{i}"][:, j:j + 1]

        def gbias(j):
            return W["gbias"][:, j:j + 1]

        # persistent node-state tiles (full shard width)
        nstate_f = nsp.tile([P, NSp], dt.float32, name="nstate_f", tag="nstate_f")
        ngate_f = nsp.tile([P, NSp], dt.float32, name="ngate_f", tag="ngate_f")
        ngate_b = nsp.tile([P, NSp], dt.bfloat16, name="ngate_b", tag="ngate_b")
        nagg_b = nsp.tile([P, NSp], dt.bfloat16, name="nagg_b", tag="nagg_b")

        # node table (bf16, DRAM) + edge inter-layer state
        table = drp.tile([NTpad, D], dt.bfloat16, name="table", tag="table")
        edge_mid = drp.tile([P, E_pad], dt.float32, name="edge_mid", tag="edge_mid")

        # ---------------- helpers ----------------
        def gru(pref, x_bf, h_bf, h_f32, out_f, sl, gb0, is_node, init=False):
            """GRU cell, f-major [D, sl] tiles; x_bf/h_bf bf16 rhs tiles.
            gb0: base col in gbias (0 node / 4 edge). Writes out_f (f32)."""
            wih = W[("gnw_ih_bf" if is_node else "gew_ih_bf")]
            whh = W[("gnw_hh_bf" if is_node else "gew_hh_bf")]
            ps_r = pbig.tile([P, sl], dt.float32, name=f"{pref}psr", tag="pbig")
            ps_z = pbig.tile([P, sl], dt.float32, name=f"{pref}psz", tag="pbig")
            ps_i = pbig.tile([P, sl], dt.float32, name=f"{pref}psi", tag="pbig")
            nc.tensor.matmul(ps_r[:], wih[:, 0:D], x_bf, start=True, stop=init)
            nc.tensor.matmul(ps_z[:], wih[:, D:2 * D], x_bf, start=True, stop=init)
            nc.tensor.matmul(ps_i[:], wih[:, 2 * D:3 * D], x_bf, start=True, stop=True)
            if not init:
                nc.tensor.matmul(ps_r[:], whh[:, 0:D], h_bf, start=False, stop=True)
                nc.tensor.matmul(ps_z[:], whh[:, D:2 * D], h_bf, start=False, stop=True)
            r = wkp.tile([P, sl], dt.float32, name=f"{pref}r", tag="gA")
            z = wkp.tile([P, sl], dt.float32, name=f"{pref}z", tag="gB")
            nc.scalar.activation(r[:], ps_r[:], AF.Sigmoid, bias=gbias(gb0 + 0))
            nc.scalar.activation(z[:], ps_z[:], AF.Sigmoid, bias=gbias(gb0 + 1))
            nin = wkp.tile([P, sl], dt.float32, name=f"{pref}nin", tag="gC")
            if init:
                # n_pre = r * b_hh_n + inn
                nc.vector.scalar_tensor_tensor(
                    out=nin[:], in0=r[:], scalar=gbias(gb0 + 2), in1=ps_i[:],
                    op0=OP.mult, op1=OP.add)
            else:
                ps_h = pbig.tile([P, sl], dt.float32, name=f"{pref}psh", tag="pbig")
                nc.tensor.matmul(ps_h[:], whh[:, 2 * D:3 * D], h_bf,
                                 start=True, stop=True)
                hn = wkp.tile([P, sl], dt.float32, name=f"{pref}hn", tag="gD")
                nc.scalar.activation(hn[:], ps_h[:], AF.Identity, bias=gbias(gb0 + 2))
                nc.vector.tensor_mul(nin[:], r[:], hn[:])
                nc.vector.tensor_add(nin[:], nin[:], ps_i[:])
            n = wkp.tile([P, sl], dt.float32, name=f"{pref}n", tag="gE")
            nc.scalar.activation(n[:], nin[:], AF.Tanh, bias=gbias(gb0 + 3))
            t2 = wkp.tile([P, sl], dt.float32, name=f"{pref}t2", tag="gD2")
            if init:
                nc.vector.tensor_mul(t2[:], z[:], n[:])       # z*n
                nc.vector.tensor_sub(out_f, n[:], t2[:])      # n - z*n
            else:
                nc.vector.tensor_sub(t2[:], h_f32, n[:])      # h - n
                nc.vector.tensor_mul(t2[:], z[:], t2[:])      # z*(h-n)
                nc.vector.tensor_add(out_f, n[:], t2[:])      # n + z*(h-n)

        def gate_and_ag(layer):
            """geo-gate nstate -> ngate; transpose to e-major; AllGather ->
            rebuild bf16 table."""
            ag_in = drp.tile([NS, D], dt.float32, name=f"agin{layer}",
                             tag=f"agin{layer}")
            ag_out = drp.tile([N, D], dt.float32, name=f"agout{layer}",
                              tag=f"agout{layer}", addr_space="Shared")
            for s in range(NSUBT):
                sl = slice(s * SUB, (s + 1) * SUB)
                ns_b = wkp.tile([P, SUB], dt.bfloat16, name=f"nsb{layer}{s}",
                                tag="nsb")
                nc.vector.tensor_copy(ns_b[:], nstate_f[:, sl])
                sg = wkp.tile([P, SUB], dt.float32, name=f"sg{layer}{s}",
                              tag="sgl")
                nc.sync.dma_start(sg[:], di["sgeoT"][:, sl])
                psg = psml.tile([1, SUB], dt.float32, name=f"psg{layer}{s}",
                                tag="psml")
                nc.tensor.matmul(psg[:], W["g0_bf"][:, 0:1], ns_b[:],
                                 start=True, stop=False)
                nc.tensor.matmul(psg[:], W["ones1"][:, 0:1],
                                 W["cgeo_bf"][:, sl], start=False, stop=True)
                gate_b = wkp.tile([1, SUB], dt.bfloat16, name=f"gb{layer}{s}",
                                  tag="gateb")
                nc.scalar.activation(gate_b[:], psg[:], AF.Sigmoid)
                psf = pbig.tile([P, SUB], dt.float32, name=f"psf{layer}{s}",
                                tag="pbig")
                nc.tensor.matmul(psf[:], W["ones1"][:], gate_b[:],
                                 start=True, stop=True)
                tmp = wkp.tile([P, SUB], dt.float32, name=f"gt{layer}{s}",
                               tag="gA")
                nc.vector.tensor_mul(tmp[:], psf[:], sg[:])
                nc.vector.tensor_add(ngate_f[:, sl], nstate_f[:, sl], tmp[:])
                nc.vector.tensor_copy(ngate_b[:, sl], ngate_f[:, sl])
                for ec in range(SUB // P):
                    base = s * SUB + ec * P
                    nrows = min(P, NS - base)
                    if nrows <= 0:
                        continue
                    pst = psml.tile([P, P], dt.float32,
                                    name=f"pst{layer}{s}{ec}", tag="psml")
                    nc.tensor.transpose(pst[:], ngate_f[:, base:base + P],
                                        ident[:])
                    em = wkp.tile([P, P], dt.float32, name=f"em{layer}{s}{ec}",
                                  tag="emc", bufs=4)
                    nc.vector.tensor_copy(em[:], pst[:])
                    nc.sync.dma_start(ag_in[base:base + nrows, :],
                                      em[:nrows, :])
            nc.gpsimd.collective_compute(
                "AllGather", OP.bypass,
                replica_groups=[list(range(NCORES))],
                ins=[ag_in[:].opt()], outs=[ag_out[:].opt()])
            nc.gpsimd.dma_start(table[0:N, :], ag_out[:])  # f32 -> bf16 cast

        # ---------------- phase A: node init + layer-0 gate + AG ----------
        for s in range(NSUBT):
            sl = slice(s * SUB, (s + 1) * SUB)
            nin_b = wkp.tile([P, SUB], dt.bfloat16, name=f"ninb{s}", tag="ninb")
            nc.gpsimd.dma_start(nin_b[:], di["nodeT_in"][:, sl])  # f32->bf16
            gru(f"ni{s}", nin_b[:], None, None, nstate_f[:, sl], SUB, 0,
                True, init=True)
        gate_and_ag(0)

        # ---------------- edge + node layers ----------------
        for li in range(L):
            relu_msg = (li < L - 1) or (L == 1)
            We1p, qkv = W[f"We1p_{li}"], W[f"qkv_{li}"]
            Wa1e, Wa2e = W[f"Wa1e_{li}"], W[f"Wa2e_{li}"]
            We2, Wu1p, Wu2 = W[f"We2_{li}"], W[f"Wu1p_{li}"], W[f"Wu2_{li}"]
            ba2e, bve = W[f"ba2e_{li}"], W[f"bve_{li}"]
            win_ps = {}

            for m in range(NMAC):
                mb = m * MACRO
                msl = slice(mb, mb + MACRO)
                xiT = emp.tile([P, MACRO], dt.bfloat16, name=f"xi{li}{m}", tag="xiT")
                xjT = emp.tile([P, MACRO], dt.bfloat16, name=f"xj{li}{m}", tag="xjT")
                idr = emp.tile([P, MACRO // 16], dt.int16, name=f"ir{li}{m}", tag="idr")
                idc = emp.tile([P, MACRO // 16], dt.int16, name=f"ic{li}{m}", tag="idc")
                rlo = emp.tile([P, MACRO // P], dt.float32, name=f"rl{li}{m}", tag="rlo")
                nc.sync.dma_start(idr[:], di["idx_row"][:, m * P:(m + 1) * P])
                nc.sync.dma_start(idc[:], di["idx_col"][:, m * P:(m + 1) * P])
                nc.sync.dma_start(rlo[:], di["rowloc"][:, m * (MACRO // P):(m + 1) * (MACRO // P)])
                src_e = di["edgeT_in"] if li == 0 else edge_mid
                GIDX = 512
                for xx, ii in ((xiT, idr), (xjT, idc)):
                    for gi in range(MACRO // GIDX):
                        nc.gpsimd.dma_gather(
                            out_ap=xx[:, gi * GIDX:(gi + 1) * GIDX].unsqueeze(1),
                            in_ap=table[:], idxs_ap=ii[:, gi * (GIDX // 16):(gi + 1) * (GIDX // 16)],
                            num_idxs=GIDX, num_idxs_reg=GIDX, elem_size=D,
                            transpose=True)

                for st in range(MACRO // SUB):
                    ssl = slice(st * SUB, (st + 1) * SUB)
                    sub0 = mb + st * SUB
                    pref = f"e{li}{m}{st}"
                    eTs = wkp.tile([P, SUB], dt.float32, name=f"{pref}et",
                                   tag="eTs")
                    nc.sync.dma_start(eTs[:], src_e[:, sub0:sub0 + SUB])
                    # edge state (bf16) + h (f32)
                    if li == 0:
                        raw_b = wkp.tile([P, SUB], dt.bfloat16, name=f"{pref}rb",
                                         tag="rawb")
                        nc.vector.tensor_copy(raw_b[:], eTs[:])
                        e0f = wkp.tile([P, SUB], dt.float32, name=f"{pref}e0f",
                                       tag="e0f")
                        gru(pref + "i", raw_b[:], None, None, e0f[:], SUB, 4,
                            False, init=True)
                        st_b = wkp.tile([P, SUB], dt.bfloat16, name=f"{pref}stb",
                                        tag="stb")
                        nc.vector.tensor_copy(st_b[:], e0f[:])
                        h_f = e0f[:]
                    else:
                        st_b = wkp.tile([P, SUB], dt.bfloat16, name=f"{pref}stb",
                                        tag="stb")
                        nc.vector.tensor_copy(st_b[:], eTs[:])
                        h_f = eTs[:]
                    xi_s, xj_s = xiT[:, ssl], xjT[:, ssl]

                    # stage1: MLP1 + q + k
                    psA = pbig.tile([P, SUB], dt.float32, name=f"{pref}psA", tag="pbig")
                    psB = pbig.tile([P, SUB], dt.float32, name=f"{pref}psB", tag="pbig")
                    for oc, ps in ((0, psA), (1, psB)):
                        nc.tensor.matmul(ps[:], We1p[:, oc * P:(oc + 1) * P], xi_s,
                                         start=True, stop=False)
                        nc.tensor.matmul(ps[:], We1p[:, 2 * D + oc * P:2 * D + (oc + 1) * P],
                                         st_b[:], start=False, stop=False)
                        nc.tensor.matmul(ps[:], We1p[:, 4 * D + oc * P:4 * D + (oc + 1) * P],
                                         xj_s, start=False, stop=True)
                    psQ = pbig.tile([P, SUB], dt.float32, name=f"{pref}psQ", tag="pbig")
                    psK = pbig.tile([P, SUB], dt.float32, name=f"{pref}psK", tag="pbig")
                    nc.tensor.matmul(psQ[:], qkv[:, 0:D], xi_s, start=True, stop=True)
                    nc.tensor.matmul(psK[:], qkv[:, D:2 * D], st_b[:], start=True, stop=True)
                    aq = wkp.tile([P, SUB], dt.bfloat16, name=f"{pref}aq", tag="aq")
                    ak = wkp.tile([P, SUB], dt.bfloat16, name=f"{pref}ak", tag="ak")
                    nc.scalar.activation(aq[:], psQ[:], AF.Identity, bias=bias(li, 3))
                    nc.scalar.activation(ak[:], psK[:], AF.Identity, bias=bias(li, 4))
                    h1_0 = wkp.tile([P, SUB], dt.bfloat16, name=f"{pref}h10", tag="h10")
                    h1_1 = wkp.tile([P, SUB], dt.bfloat16, name=f"{pref}h11", tag="h11")
                    nc.scalar.activation(h1_0[:], psA[:], AF.Relu, bias=bias(li, 0))
                    nc.scalar.activation(h1_1[:], psB[:], AF.Relu, bias=bias(li, 1))

                    # stage2: attention hidden
                    ha0 = wkp.tile([P, SUB], dt.bfloat16, name=f"{pref}ha0", tag="ha0")
                    ha1 = wkp.tile([P, SUB], dt.bfloat16, name=f"{pref}ha1", tag="ha1")
                    for oc, ha in ((0, ha0), (1, ha1)):
                        psH = pbig.tile([P, SUB], dt.float32, name=f"{pref}psH{oc}",
                                        tag="pbig")
                        nc.tensor.matmul(psH[:], Wa1e[0][:, oc * P:(oc + 1) * P], aq[:],
                                         start=True, stop=False)
                        nc.tensor.matmul(psH[:], Wa1e[1][:, oc * P:(oc + 1) * P], ak[:],
                                         start=False, stop=True)
                        nc.scalar.activation(ha[:], psH[:], AF.Relu, bias=bias(li, 5 + oc))

                    # MLP2 -> edge_msg
                    psM = pbig.tile([P, SUB], dt.float32, name=f"{pref}psM", tag="pbig")
                    nc.tensor.matmul(psM[:], We2[0][:], h1_0[:], start=True, stop=False)
                    nc.tensor.matmul(psM[:], We2[1][:], h1_1[:], start=False, stop=True)
                    msg_b = wkp.tile([P, SUB], dt.bfloat16, name=f"{pref}msg", tag="msg")
                    nc.scalar.activation(msg_b[:], psM[:],
                                         AF.Relu if relu_msg else AF.Identity,
                                         bias=bias(li, 2))

                    # per-128-edge chunk: attention scores, softmax, value, seg
                    for cchunk in range(SUB // P):
                        cs_sub = slice(cchunk * P, (cchunk + 1) * P)
                        cs_mac = slice(st * SUB + cchunk * P,
                                       st * SUB + (cchunk + 1) * P)
                        kchunk = (sub0 + cchunk * P) // P   # global chunk idx
                        w = kchunk // KW                    # window (uniform)
                        cp = f"{pref}c{cchunk}"
                        psT = psml.tile([P, P], dt.float32, name=f"{cp}at", tag="psml")
                        nc.tensor.matmul(psT[:], ha0[:, cs_sub], Wa2e[0][:],
                                         start=True, stop=False)
                        nc.tensor.matmul(psT[:], ha1[:, cs_sub], Wa2e[1][:],
                                         start=False, stop=False)
                        nc.tensor.matmul(psT[:], W["ones1"][:], ba2e[:],
                                         start=False, stop=True)
                        ex = ckp.tile([P, P], dt.float32, name=f"{cp}ex", tag="ex")
                        nc.scalar.activation(ex[:], psT[:], AF.Exp)
                        sm = ckp.tile([P, H], dt.float32, name=f"{cp}sm", tag="sm")
                        ex3 = ex[:].rearrange("p (h c) -> p h c", h=H)
                        nc.vector.tensor_reduce(sm[:], ex3, mybir.AxisListType.X, OP.add)
                        nc.vector.reciprocal(sm[:], sm[:])
                        prob = ckp.tile([P, P], dt.float32, name=f"{cp}pr", tag="pr")
                        nc.vector.tensor_tensor(
                            out=prob[:].rearrange("p (h c) -> p h c", h=H),
                            in0=ex3,
                            in1=sm[:, :, None].broadcast_to([P, H, D // H]),
                            op=OP.mult)
                        nc.sync.dma_start(probs_o[li][(sub0 // P) + cchunk], prob[:])
                        psV = psml.tile([P, P], dt.float32, name=f"{cp}v", tag="psml")
                        nc.tensor.matmul(psV[:], xjT[:, cs_mac], qkv[:, 2 * D:3 * D],
                                         start=True, stop=False)
                        nc.tensor.matmul(psV[:], W["ones1"][:], bve[:],
                                         start=False, stop=True)
                        val = ckp.tile([P, P], dt.bfloat16, name=f"{cp}vl", tag="vl")
                        nc.vector.tensor_mul(val[:], prob[:], psV[:])
                        if w < NWIN:
                            seg = ckp.tile([P, P], dt.bfloat16, name=f"{cp}sg", tag="sg")
                            nc.vector.scalar_tensor_tensor(
                                out=seg[:],
                                in0=rlo[:, st * (SUB // P) + cchunk:st * (SUB // P) + cchunk + 1]
                                    .to_broadcast([P, P]),
                                scalar=float(-(w * P)), in1=W["iota"][:],
                                op0=OP.add, op1=OP.is_equal)
                            first = (kchunk % KW == 0)
                            last = (kchunk % KW == KW - 1)
                            if first:
                                win_ps[w] = pwin.tile([P, P], dt.float32,
                                                      name=f"w{li}_{w}", tag="pwin")
                            nc.tensor.matmul(win_ps[w][:], seg[:], val[:],
                                             start=first, stop=last,
                                             skip_group_check=True)
                            if last:
                                agg_e = wkp.tile([P, P], dt.float32,
                                                 name=f"ag{li}{w}", tag="emc", bufs=4)
                                nc.vector.tensor_copy(agg_e[:], win_ps[w][:])
                                pst = psml.tile([P, P], dt.float32,
                                                name=f"wt{li}{w}", tag="psml")
                                nc.tensor.transpose(pst[:], agg_e[:], ident[:])
                                nc.vector.tensor_copy(
                                    nagg_b[:, w * P:(w + 1) * P], pst[:])

                    # edge GRU -> new state
                    eo = wkp.tile([P, SUB], dt.float32, name=f"{pref}eo", tag="eo")
                    gru(pref + "g", msg_b[:], st_b[:], h_f, eo[:], SUB, 4, False)
                    dst_e = edge_mid if li == 0 else edge_out
                    nc.sync.dma_start(dst_e[:, sub0:sub0 + SUB], eo[:])

            # ---- node update for this layer ----
            for s in range(NSUBT):
                sl = slice(s * SUB, (s + 1) * SUB)
                pref = f"n{li}{s}"
                psU0 = pbig.tile([P, SUB], dt.float32, name=f"{pref}u0", tag="pbig")
                psU1 = pbig.tile([P, SUB], dt.float32, name=f"{pref}u1", tag="pbig")
                for oc, ps in ((0, psU0), (1, psU1)):
                    nc.tensor.matmul(ps[:], Wu1p[0][:, oc * P:(oc + 1) * P],
                                     ngate_b[:, sl], start=True, stop=False)
                    nc.tensor.matmul(ps[:], Wu1p[1][:, oc * P:(oc + 1) * P],
                                     nagg_b[:, sl], start=False, stop=True)
                hu0 = wkp.tile([P, SUB], dt.bfloat16, name=f"{pref}hu0", tag="h10")
                hu1 = wkp.tile([P, SUB], dt.bfloat16, name=f"{pref}hu1", tag="h11")
                nc.scalar.activation(hu0[:], psU0[:], AF.Relu, bias=bias(li, 7))
                nc.scalar.activation(hu1[:], psU1[:], AF.Relu, bias=bias(li, 8))
                psN = pbig.tile([P, SUB], dt.float32, name=f"{pref}pn", tag="pbig")
                nc.tensor.matmul(psN[:], Wu2[0][:], hu0[:], start=True, stop=False)
                nc.tensor.matmul(psN[:], Wu2[1][:], hu1[:], start=False, stop=True)
                nmsg = wkp.tile([P, SUB], dt.bfloat16, name=f"{pref}nm", tag="msg")
                nc.scalar.activation(nmsg[:], psN[:],
                                     AF.Relu if relu_msg else AF.Identity,
                                     bias=bias(li, 9))
                gru(pref + "g", nmsg[:], ngate_b[:, sl], ngate_f[:, sl],
                    nstate_f[:, sl], SUB, 0, True)
                if li == L - 1:
                    nc.sync.dma_start(node_out[:, sl], nstate_f[:, sl])
            if li < L - 1:
                gate_and_ag(li + 1)

    nc.compile()
    return nc


# ----------------------------------------------------------------------------
# public entry
# ----------------------------------------------------------------------------

def kernel(node, edge, geo_feature, edge_index,
           gnw_ih, gnw_hh, gnb_ih, gnb_hh,
           gew_ih, gew_hh, geb_ih, geb_hh,
           geo_w, geo_b,
           We1, be1, We2, be2,
           Wq, bq, Wk, bk, Wv, bv,
           Wa1, ba1, Wa2, ba2,
           Wu1, bu1, Wu2, bu2):
    from concourse import bass_utils

    wdict = dict(gnw_ih=gnw_ih, gnw_hh=gnw_hh, gnb_ih=gnb_ih, gnb_hh=gnb_hh,
                 gew_ih=gew_ih, gew_hh=gew_hh, geb_ih=geb_ih, geb_hh=geb_hh,
                 geo_w=geo_w, geo_b=geo_b, We1=We1, be1=be1, We2=We2, be2=be2,
                 Wq=Wq, bq=bq, Wk=Wk, bk=bk, Wv=Wv, bv=bv,
                 Wa1=Wa1, ba1=ba1, Wa2=Wa2, ba2=ba2,
                 Wu1=Wu1, bu1=bu1, Wu2=Wu2, bu2=bu2)
    wdict = {k: np.asarray(v, F32) for k, v in wdict.items()}
    node = np.asarray(node, F32)
    edge = np.asarray(edge, F32)
    geo_feature = np.asarray(geo_feature, F32)
    edge_index = np.asarray(edge_index)

    L = We1.shape[0]
    cores, meta = _prep(node, edge, geo_feature, edge_index, wdict)
    wdev = _prep_weights(wdict, meta)

    nc = _build(meta, L)

    in_maps = []
    for c in range(NCORES):
        m = {k: v for k, v in cores[c].items() if not k.startswith("_")}
        m.update(wdev)
        in_maps.append(m)

    trace = os.environ.get("GNN_TRACE", "0") == "1"
    import time as _time
    t0 = _time.time()
    try:
        res = bass_utils.run_bass_kernel_spmd(
            nc, in_maps, core_ids=list(range(NCORES)), trace=trace)
    except ModuleNotFoundError:
        res = bass_utils.run_bass_kernel_spmd(
            nc, in_maps, core_ids=list(range(NCORES)), trace=False)
    wall_ns = int((_time.time() - t0) * 1e9)
    kernel.last_exec_ns = res.exec_time_ns if res.exec_time_ns else wall_ns

    # ---- unshard ----
    N, D = node.shape
    E = edge.shape[0]
    NS, E_pad = meta["NS"], meta["E_pad"]
    dnp = D // H
    perm = meta["perm"]

    node_full = np.empty((N, D), F32)
    edge_full = np.empty((E, D), F32)
    probs_full = np.empty((L, E, dnp, H), F32)
    for c in range(NCORES):
        r = res.results[c]
        node_full[c * NS:(c + 1) * NS] = r["node_out"].T[:NS]
        sel = cores[c]["_sel"]
        valid = cores[c]["_valid"]
        src = cores[c]["_src"][valid]          # padded-slot -> local sorted idx
        orig = sel[src]                        # original edge ids
        eT = r["edge_out"][:, valid]           # [D, n_real]
        edge_full[orig] = eT.T
        for li in range(L):
            pr = r[f"probs{li}"].reshape(E_pad, P)[valid]   # [n_real, 128] g-major
            probs_full[li][orig] = pr.reshape(-1, H, dnp).transpose(0, 2, 1)
    return node_full, edge_full, probs_full
